# revision 17
# baseline (speedup 1.0000x reference)
"""AxonalConnections GNN message passing on 8 TRN2 NeuronCores.

out[n] = sum_{e: dst[e]==n} spikes[src[e]] * masks[src[e]] * weights[e]

Sharding: H dim (1024) split across 8 cores -> per-core shard has exactly
128 h-rows = SBUF partition count. Pure data parallel (edges replicated),
no collectives.

Per core: partition dim = h, free dim = w in 8 chunks of 128.
Division of labor (each engine on its own SBUF ports, no contention):
  DVE     wm = masks*weights (in place), sig = spikes*wm   (bf16, 2x TT)
  PE      segment-sum: 4 identity matmuls per node accumulate the node's
          incoming sig slots into one PSUM bank (f32 accumulate)
  ScalarE PSUM -> SBUF drain with f32->bf16 cast
  DMA     packed contiguous loads / stores (HWDGE)
spikes {0,1} and masks {1,-0.5} are exact in bf16 and the reduction is
f32, so only weight rounding + the final bf16 store round. Inputs are
host-packed into one contiguous per-chunk array so every load is one
large linear DMA. sig is double-buffered: PE consumes chunk c while the
DVE builds chunk c+1.
"""

import numpy as np
import ml_dtypes

import concourse.bacc as bacc
import concourse.mybir as mybir
import concourse.tile as tile
from concourse.bass_utils import run_bass_kernel_spmd

# Problem shape (hardcoded per spec)
N_NODES = 8
N_EDGES = 32
B = 4
H = 1024
W = 1024
N_CORES = 8
H_SH = H // N_CORES  # 128 = partition count
F = 128              # w-chunk size (also the pipelining granule)
N_CHUNK = W // F
NQ = 4               # load/compute quarters per chunk (2 sources each)
NSQ = N_NODES // NQ  # 2
KDEG = N_EDGES // N_NODES  # 4

# packed per-(chunk,h,quarter) input block: spikes | masks | weights
SP_LEN = NSQ * B * F          # 1024
MK_LEN = NSQ * F              # 256
WT_LEN = NSQ * KDEG * F       # 1024
QB_LEN = SP_LEN + MK_LEN + WT_LEN  # 2304

BF16 = mybir.dt.bfloat16
NP_BF16 = ml_dtypes.bfloat16


def _edge_plan(src, dst):
    """Sort edges by src; return (perm, groups, incoming)."""
    E = len(src)
    perm = sorted(range(E), key=lambda e: (src[e], e))
    groups = []
    j = 0
    while j < E:
        s = src[perm[j]]
        j0 = j
        while j < E and src[perm[j]] == s:
            j += 1
        groups.append((s, j0, [perm[t] for t in range(j0, j)]))
    incoming = [[] for _ in range(N_NODES)]
    for j, e in enumerate(perm):
        incoming[dst[e]].append(j)
    return perm, groups, incoming


def _is_ring(src, dst):
    """True iff the edge list is the ring i -> (i+1..i+4) mod 8."""
    perm, _, _ = _edge_plan(src, dst)
    for s in range(N_NODES):
        for t in range(KDEG):
            e = perm[4 * s + t]
            if src[e] != s or dst[e] != (s + 1 + t) % N_NODES:
                return False
    return True


def _stride_runs(triples):
    """Split (o, a, b) index triples into maximal runs where all three
    sequences advance with a constant positive stride."""
    runs = []
    i = 0
    while i < len(triples):
        j = i + 1
        if j < len(triples):
            d = tuple(triples[j][t] - triples[i][t] for t in range(3))
            if all(x > 0 for x in d):
                while j < len(triples) and all(
                    triples[j][t] - triples[j - 1][t] == d[t] for t in range(3)
                ):
                    j += 1
            else:
                d = None
        else:
            d = None
        if j == i + 1:
            runs.append((i, 1, (1, 1, 1)))
        else:
            runs.append((i, j - i, d))
        i = j
    return runs


def _slot_view(view, start, count, stride):
    """AP view over dim-1 with the given stride."""
    if count == 1:
        return view[:, start : start + 1]
    return view[:, start : start + (count - 1) * stride + 1 : stride]


def _ring_l1_l2():
    """Level-1 pairings and level-2 triples for the ring structure.

    Slot of edge (s -> s+1+t) is 4s+t. For dst n, pair (t=0,t=2) and
    (t=1,t=3) so both pair streams and the level-2 combine advance with
    constant stride across n (mod wraps give ~3 runs each).
    L1A: slot a_n += slot c_n ; L1B: slot b_n += slot d_n
    L2 : out[n] = slot a_n + slot b_n
    """
    A, Bp, L2 = [], [], []
    for n in range(N_NODES):
        a = (4 * ((n - 1) % 8)) % 32
        c = (4 * ((n - 3) % 8) + 2) % 32
        b = (4 * ((n - 2) % 8) + 1) % 32
        d = (4 * ((n - 4) % 8) + 3) % 32
        A.append((a, a, c))
        Bp.append((b, b, d))
        L2.append((n, a, b))
    return A, Bp, L2


def _split_l1_runs(gps_target):
    """Assign L1 stride-runs greedily (longest first) to GpSimd until it
    holds >= gps_target pairs; the rest go to the DVE.
    Returns (gps_ops, dve_ops) as lists of (triples, i0, cnt, d)."""
    A, Bp, _ = _ring_l1_l2()
    all_runs = []
    for trips in (A, Bp):
        for i0, cnt, d in _stride_runs(trips):
            all_runs.append((trips, i0, cnt, d))
    all_runs.sort(key=lambda r: -r[2])
    gps, dve, got = [], [], 0
    for r in all_runs:
        if got < gps_target:
            gps.append(r)
            got += r[2]
        else:
            dve.append(r)
    return gps, dve


def _ring_slots(n):
    """Incoming sig slots (sorted-by-src order) for dst n in the ring."""
    return [(4 * ((n - 1 - t) % 8) + t) % 32 for t in range(KDEG)]


def _build_ring(nc):
    """Optimized program for the ring edge structure."""
    from concourse import masks as bass_masks

    inp = nc.dram_tensor(
        "inp", [N_CHUNK, H_SH, NQ, QB_LEN], BF16, kind="ExternalInput"
    ).ap()
    outd = nc.dram_tensor(
        "out", [N_CHUNK, H_SH, N_NODES, B, F], BF16, kind="ExternalOutput"
    ).ap()

    F32 = mybir.dt.float32

    def views(in_t, q):
        base = in_t[:, q]
        sp = base[:, 0:SP_LEN].rearrange("p (s b f) -> p s b f", s=NSQ, b=B)
        mk = base[:, SP_LEN : SP_LEN + MK_LEN].rearrange("p (s f) -> p s f", s=NSQ)
        wt = base[:, SP_LEN + MK_LEN : QB_LEN].rearrange(
            "p (s k f) -> p s k f", s=NSQ, k=KDEG
        )
        return sp, mk, wt

    with tile.TileContext(nc) as tc:
        with (
            tc.tile_pool(name="consts", bufs=1) as consts,
            tc.tile_pool(name="in", bufs=2) as in_pool,
            tc.tile_pool(name="sig", bufs=2) as sig_pool,
            tc.tile_pool(name="out", bufs=4) as out_pool,
            tc.tile_pool(name="psum", bufs=2, space="PSUM") as psum_pool,
        ):
            identity = consts.tile([H_SH, H_SH], BF16, name="identity")
            bass_masks.make_identity(nc, identity[:])

            in_ts = [None] * N_CHUNK
            sig_ts = [None] * N_CHUNK

            def emit_load(c, quarters):
                in_ts[c] = in_pool.tile(
                    [H_SH, NQ, QB_LEN], BF16, tag="in", name=f"in{c}"
                )
                if quarters:
                    nc.sync.dma_start(
                        out=in_ts[c][:, 0, SP_LEN:], in_=inp[c, :, 0, SP_LEN:]
                    )
                    nc.sync.dma_start(
                        out=in_ts[c][:, 0, 0:SP_LEN], in_=inp[c, :, 0, 0:SP_LEN]
                    )
                    for q in range(1, NQ):
                        nc.sync.dma_start(out=in_ts[c][:, q], in_=inp[c, :, q])
                else:
                    nc.sync.dma_start(out=in_ts[c][:], in_=inp[c])

            def emit_wm(c):
                # wm[e] = masks[src_e] * w[e], in place (masks are powers of
                # two so this rounds nothing); per quarter so every AP stays
                # within 3 free dims (walrus TENSOR3D limit, bcast dims count)
                for q in range(NQ):
                    _, mk, wt = views(in_ts[c], q)
                    nc.vector.tensor_mul(
                        out=wt,
                        in0=wt,
                        in1=mk[:, :, None, :].broadcast_to(
                            [H_SH, NSQ, KDEG, F]
                        ),
                    )

            CSP, CMK = NQ * SP_LEN, NQ * MK_LEN

            def cviews(in_t):
                flat = in_t.rearrange("p q l -> p (q l)")
                spv = flat[:, 0:CSP].rearrange(
                    "p (s b f) -> p s b f", s=N_NODES, b=B
                )
                mkv = flat[:, CSP : CSP + CMK].rearrange(
                    "p (s f) -> p s f", s=N_NODES
                )
                wtv = flat[:, CSP + CMK :].rearrange(
                    "p (s k f) -> p s k f", s=N_NODES, k=KDEG
                )
                return spv, mkv, wtv

            def emit_wm_fused(c):
                _, mkv, wtv = cviews(in_ts[c])
                nc.vector.tensor_mul(
                    out=wtv,
                    in0=wtv,
                    in1=mkv[:, :, None, :].broadcast_to(
                        [H_SH, N_NODES, KDEG, F]
                    ),
                )

            def emit_sig_fused(c):
                sig_ts[c] = sig_pool.tile(
                    [H_SH, N_EDGES, B, F], BF16, tag="sig", name=f"sig{c}"
                )
                spv, _, wtv = cviews(in_ts[c])
                nc.vector.tensor_mul(
                    out=sig_ts[c].rearrange(
                        "p (s k) b f -> p s k b f", k=KDEG
                    ),
                    in0=spv[:, :, None].broadcast_to(
                        [H_SH, N_NODES, KDEG, B, F]
                    ),
                    in1=wtv[:, :, :, None].broadcast_to(
                        [H_SH, N_NODES, KDEG, B, F]
                    ),
                )

            def emit_sig(c):
                sig_ts[c] = sig_pool.tile(
                    [H_SH, N_EDGES, B, F], BF16, tag="sig", name=f"sig{c}"
                )
                for q in range(NQ):
                    sp, _, wt = views(in_ts[c], q)
                    nc.vector.tensor_mul(
                        out=sig_ts[c][:, q * 8 : q * 8 + 8].rearrange(
                            "p (s k) b f -> p s k b f", k=KDEG
                        ),
                        in0=sp[:, :, None].broadcast_to(
                            [H_SH, NSQ, KDEG, B, F]
                        ),
                        in1=wt[:, :, :, None].broadcast_to(
                            [H_SH, NSQ, KDEG, B, F]
                        ),
                    )

            def emit_reduce(c, g0, gn):
                # PE: out[n] = sum of its 4 incoming sig slots, one identity
                # matmul per slot accumulating into node n's PSUM bank
                sig_m = sig_ts[c].rearrange("p s b f -> p s (b f)")
                ps = psum_pool.tile(
                    [H_SH, gn, B * F], F32, tag="ps", name=f"ps{c}_{g0}"
                )
                for ni in range(gn):
                    slots = _ring_slots(g0 + ni)
                    for k, slot in enumerate(slots):
                        nc.tensor.matmul(
                            ps[:, ni],
                            identity[:],
                            sig_m[:, slot],
                            start=(k == 0),
                            stop=(k == KDEG - 1),
                        )
                # ScalarE: PSUM -> SBUF with f32 -> bf16 cast
                ot = out_pool.tile(
                    [H_SH, gn, B * F], BF16, tag="out", name=f"ot{c}_{g0}"
                )
                nc.scalar.copy(out=ot[:], in_=ps[:])
                nc.sync.dma_start(
                    out=outd[c, :, g0 : g0 + gn],
                    in_=ot.rearrange("p n (b f) -> p n b f", b=B),
                )

            def emit_dve_reduce(c, nodes):
                # endgame: the DVE is idle after the last sig, so it reduces
                # these nodes itself. Pair sums go to a DVE-private scratch
                # tile (not in place) so the strided writes cannot alias the
                # PE's concurrent slot reads in the dependency tracker.
                A, Bp, L2T = _ring_l1_l2()
                V = sig_ts[c].rearrange("p s b f -> p s (b f)")
                nn = len(nodes)
                tmp = out_pool.tile(
                    [H_SH, 2 * nn, B * F], BF16, tag="out", name=f"tmp{c}"
                )
                ot = out_pool.tile(
                    [H_SH, nn, B * F], BF16, tag="out", name=f"otd{c}"
                )
                for half, trips in enumerate((A, Bp)):
                    sub = [
                        (half * nn + i, trips[n][1], trips[n][2])
                        for i, n in enumerate(nodes)
                    ]
                    for i0, cnt, d in _stride_runs(sub):
                        t0, a0, b0 = sub[i0]
                        nc.vector.tensor_add(
                            out=_slot_view(tmp, t0, cnt, d[0]),
                            in0=_slot_view(V, a0, cnt, d[1]),
                            in1=_slot_view(V, b0, cnt, d[2]),
                        )
                nc.vector.tensor_add(
                    out=ot[:], in0=tmp[:, 0:nn], in1=tmp[:, nn : 2 * nn]
                )
                nc.sync.dma_start(
                    out=outd[c, :, nodes[0] : nodes[0] + len(nodes)],
                    in_=ot.rearrange("p n (b f) -> p n b f", b=B),
                )

            emit_load(0, quarters=True)
            emit_wm(0)
            emit_sig(0)
            for c in range(N_CHUNK):
                last = c == N_CHUNK - 1
                if not last:
                    emit_load(c + 1, quarters=False)
                    emit_wm_fused(c + 1)
                    emit_sig_fused(c + 1)
                    for g0 in (0, 4):
                        emit_reduce(c, g0, 4)
                else:
                    # endgame: PE node-groups ordered by readiness (node n
                    # needs sources n-4..n-1, i.e. node 4 is complete after
                    # sig quarter 1, nodes 5-6 after quarter 2); the DVE
                    # picks up nodes 1-3 itself once the last sig op retires
                    emit_reduce(c, 4, 1)
                    emit_reduce(c, 5, 2)
                    emit_reduce(c, 7, 1)
                    emit_dve_reduce(c, [1, 2, 3])
                    emit_reduce(c, 0, 1)
    return outd


# ---------------------------------------------------------------------------
# Generic fallback (any src/dst): straightforward bf16 DVE-only pipeline with
# per-tensor transposed loads. Only used when the edge list is not the ring.
# ---------------------------------------------------------------------------

def _contig_runs(idxs):
    runs = []
    start = 0
    for i in range(1, len(idxs) + 1):
        if i == len(idxs) or idxs[i] != idxs[i - 1] + 1:
            runs.append((start, i))
            start = i
    return runs


def _build_generic(nc, src, dst):
    GF = 256
    GN = W // GF
    sp = nc.dram_tensor("spikes", [N_NODES, B, H_SH, W], BF16, kind="ExternalInput").ap()
    mk = nc.dram_tensor("masks", [N_NODES, H_SH, W], BF16, kind="ExternalInput").ap()
    wt = nc.dram_tensor("weights", [N_EDGES, H_SH, W], BF16, kind="ExternalInput").ap()
    out = nc.dram_tensor("out", [N_NODES, B, H_SH, W], BF16, kind="ExternalOutput").ap()

    _, groups, incoming = _edge_plan(src, dst)

    with tile.TileContext(nc) as tc:
        with (
            tc.tile_pool(name="spikes", bufs=2) as spikes_pool,
            tc.tile_pool(name="masks", bufs=2) as masks_pool,
            tc.tile_pool(name="w", bufs=2) as w_pool,
            tc.tile_pool(name="mod", bufs=1) as mod_pool,
            tc.tile_pool(name="sig", bufs=1) as sig_pool,
            tc.tile_pool(name="out", bufs=2) as out_pool,
        ):
            for c in range(GN):
                fw = GF
                wsl = slice(c * GF, (c + 1) * GF)
                mt = masks_pool.tile([H_SH, N_NODES, fw], BF16, tag="masks")
                nc.sync.dma_start(
                    out=mt[:], in_=mk[:, :, wsl].transpose([1, 0, 2])
                )
                st = spikes_pool.tile([H_SH, N_NODES, B, fw], BF16, tag="spikes")
                nc.sync.dma_start(
                    out=st[:], in_=sp[:, :, :, wsl].transpose([2, 0, 1, 3])
                )
                wtile = w_pool.tile([H_SH, N_EDGES, fw], BF16, tag="w")
                nc.sync.dma_start(
                    out=wtile[:], in_=wt[:, :, wsl].transpose([1, 0, 2])
                )

                sig_t = sig_pool.tile([H_SH, N_EDGES, B, fw], BF16)
                mod_t = mod_pool.tile([H_SH, N_NODES, B, fw], BF16, tag="mod")
                nc.vector.tensor_mul(
                    out=mod_t[:],
                    in0=st[:],
                    in1=mt[:, :, None, :].broadcast_to([H_SH, N_NODES, B, fw]),
                )
                for s, j0, edges in groups:
                    for r0, r1 in _contig_runs(edges):
                        k = r1 - r0
                        e0 = edges[r0]
                        nc.vector.tensor_mul(
                            out=sig_t[:, j0 + r0 : j0 + r1],
                            in0=mod_t[:, s][:, None].broadcast_to([H_SH, k, B, fw]),
                            in1=wtile[:, e0 : e0 + k][:, :, None].broadcast_to(
                                [H_SH, k, B, fw]
                            ),
                        )

                out_t = out_pool.tile([H_SH, N_NODES, B, fw], BF16, tag="out")
                for n in range(N_NODES):
                    slots = incoming[n]
                    if not slots:
                        nc.vector.memset(out_t[:, n], 0.0)
                        continue
                    if len(slots) == 1:
                        nc.vector.tensor_copy(out=out_t[:, n], in_=sig_t[:, slots[0]])
                        continue
                    cur = list(slots)
                    while len(cur) > 2:
                        nxt = []
                        for i in range(0, len(cur) - 1, 2):
                            a, b = cur[i], cur[i + 1]
                            nc.vector.tensor_add(
                                out=sig_t[:, a], in0=sig_t[:, a], in1=sig_t[:, b]
                            )
                            nxt.append(a)
                        if len(cur) % 2:
                            nxt.append(cur[-1])
                        cur = nxt
                    nc.vector.tensor_add(
                        out=out_t[:, n], in0=sig_t[:, cur[0]], in1=sig_t[:, cur[1]]
                    )
                nc.sync.dma_start(
                    out=out[:, :, :, wsl].transpose([2, 0, 1, 3]), in_=out_t[:]
                )
    return out


def _trace_and_compile(src, dst):
    nc = bacc.Bacc(
        "TRN2",
        target_bir_lowering=False,
        debug=False,
        num_devices=N_CORES,
    )
    if _is_ring(src, dst):
        _build_ring(nc)
    else:
        _build_generic(nc, src, dst)
    nc.compile()
    return nc


def _make_in_maps(spikes, masks, weights, src, dst):
    """Cast to bf16, H-shard across cores, pack per-chunk contiguous."""
    spikes = np.asarray(spikes).astype(NP_BF16)
    masks = np.asarray(masks).astype(NP_BF16)
    weights = np.asarray(weights).astype(NP_BF16)
    ring = _is_ring(src, dst)
    perm, _, _ = _edge_plan(src, dst)
    in_maps = []
    for i in range(N_CORES):
        hsl = slice(i * H_SH, (i + 1) * H_SH)
        spc = np.ascontiguousarray(spikes[:, :, hsl, :])
        mkc = np.ascontiguousarray(masks[:, hsl, :])
        wtc = np.ascontiguousarray(weights[:, hsl, :])
        if not ring:
            in_maps.append({"spikes": spc, "masks": mkc, "weights": wtc})
            continue
        # chunk 0: [h, q, (s2 b f | s2 f | s2 k f)] quarter blocks for a
        # fine-grained ramp; chunks 1+: [h, (s b f | s f | s k f)] whole-chunk
        # blocks so (s k) merge lets wm/sig be one DVE op per chunk
        sp6 = spc.reshape(N_NODES, B, H_SH, N_CHUNK, F).transpose(3, 2, 0, 1, 4)
        mk5 = mkc.reshape(N_NODES, H_SH, N_CHUNK, F).transpose(2, 1, 0, 3)
        wt5 = wtc[perm].reshape(N_EDGES, H_SH, N_CHUNK, F).transpose(2, 1, 0, 3)
        inp = np.empty((N_CHUNK, H_SH, NQ * QB_LEN), dtype=NP_BF16)
        inp[0] = np.concatenate(
            [
                sp6[0].reshape(H_SH, NQ, SP_LEN),
                mk5[0].reshape(H_SH, NQ, MK_LEN),
                wt5[0].reshape(H_SH, NQ, WT_LEN),
            ],
            axis=2,
        ).reshape(H_SH, NQ * QB_LEN)
        for c in range(1, N_CHUNK):
            inp[c] = np.concatenate(
                [
                    sp6[c].reshape(H_SH, NQ * SP_LEN),
                    mk5[c].reshape(H_SH, NQ * MK_LEN),
                    wt5[c].reshape(H_SH, NQ * WT_LEN),
                ],
                axis=1,
            )
        in_maps.append({"inp": np.ascontiguousarray(inp)})
    return in_maps


def _unpack_out(res, ring):
    out = np.empty((N_NODES, B, H, W), dtype=np.float32)
    for i in range(N_CORES):
        hsl = slice(i * H_SH, (i + 1) * H_SH)
        o = res.results[i]["out"].astype(np.float32)
        if ring:
            # [c, h, n, b, f] -> [n, b, h, w]
            o = o.transpose(2, 3, 1, 0, 4).reshape(N_NODES, B, H_SH, W)
        out[:, :, hsl, :] = o
    return out


def kernel(spikes, masks, weights, src_idx, dst_idx, trace=False):
    src = [int(x) for x in np.asarray(src_idx).ravel()]
    dst = [int(x) for x in np.asarray(dst_idx).ravel()]
    assert np.asarray(spikes).shape == (N_NODES, B, H, W)
    assert np.asarray(masks).shape == (N_NODES, H, W)
    assert np.asarray(weights).shape == (N_EDGES, H, W)
    assert len(src) == N_EDGES and len(dst) == N_EDGES

    nc = _trace_and_compile(src, dst)
    in_maps = _make_in_maps(spikes, masks, weights, src, dst)

    res = run_bass_kernel_spmd(
        nc, in_maps, core_ids=list(range(N_CORES)), trace=trace
    )

    out = _unpack_out(res, _is_ring(src, dst))

    if trace:
        kernel.last_exec_time_ns = res.exec_time_ns
        kernel.last_results = res
    return out
